# revision 4
# baseline (speedup 1.0000x reference)
"""TRN2 Bass kernel for nn_CudaSafeLinear: out = input @ weight.T + bias.

Shapes: input [8192, 4096] f32, weight [4096, 4096] f32, bias [4096] f32.
Sharding: data-parallel over batch rows — core c computes rows [1024c, 1024(c+1)).

Per-core GEMM (out^T orientation):
  outT[n, m] = sum_k wT[k, n] * xT[k, m] + bias[n]
with wT = weight.T ([K, N] in DRAM), xT = input_shard.T ([K, 1024]).
Stationary operand = wT k-tiles [128, 128]; moving operand = resident xT
chunks [128, 512]. Matmuls run in float32r (TF32-class precision, full PE
rate at moving dim >= 256). Accumulation is fp32 in PSUM; bias is added on
the Scalar engine during PSUM->SBUF eviction (psum partitions = out
features, so bias is a per-partition scalar).
"""

import numpy as np

import concourse.mybir as mybir
import concourse.tile as tile
from concourse import bacc
from concourse.bass_utils import run_bass_kernel_spmd

B, K, N = 8192, 4096, 4096
NCORES = 8
BC = B // NCORES          # 1024 batch rows per core
P = 128
KT = K // P               # 32 contraction tiles
MCH = BC // 512           # 2 moving chunks of 512
NSUB = N // P             # 32 stationary (out-feature) tiles
F32R = mybir.dt.float32r
F32 = mybir.dt.float32

_cached = {}


def build():
    nc = bacc.Bacc("TRN2", target_bir_lowering=False, debug=False, num_devices=NCORES)
    xT = nc.dram_tensor("xT", [K, BC], F32R, kind="ExternalInput").ap()
    wT = nc.dram_tensor("wT", [K, N], F32R, kind="ExternalInput").ap()
    bias = nc.dram_tensor("bias", [N, 1], F32, kind="ExternalInput").ap()
    outT = nc.dram_tensor("outT", [N, BC], F32, kind="ExternalOutput").ap()

    with tile.TileContext(nc) as tc:
        with (
            tc.tile_pool(name="xres", bufs=1) as x_pool,
            tc.tile_pool(name="bres", bufs=1) as b_pool,
            tc.tile_pool(name="w", bufs=8) as w_pool,
            tc.tile_pool(name="ps", bufs=8, space="PSUM") as ps_pool,
            tc.tile_pool(name="ev", bufs=4) as ev_pool,
        ):
            # Resident input shard: 32 k-tiles of [128, 1024] f32r (16.8 MB).
            x_tiles = []
            for k in range(KT):
                xt = x_pool.tile([P, BC], F32R, tag=f"x{k}")
                nc.sync.dma_start(xt[:], xT[k * P:(k + 1) * P, :])
                x_tiles.append(xt)
            # Resident bias: [128, 1] per out-feature tile.
            b_tiles = []
            for i in range(NSUB):
                bt = b_pool.tile([P, 1], F32, tag=f"b{i}")
                nc.sync.dma_start(bt[:], bias[i * P:(i + 1) * P, :])
                b_tiles.append(bt)

            for pair in range(NSUB // 2):  # two n_subs share each weight DMA
                psums = [
                    [ps_pool.tile([P, 512], F32, tag="ps", name="ps") for _ in range(MCH)]
                    for _ in range(2)
                ]
                for k in range(KT):
                    wt = w_pool.tile([P, 256], F32R, tag="w")
                    nc.sync.dma_start(
                        wt[:], wT[k * P:(k + 1) * P, 256 * pair:256 * (pair + 1)]
                    )
                    for j2 in range(2):
                        for j in range(MCH):
                            nc.tensor.matmul(
                                psums[j2][j][:],
                                wt[:, 128 * j2:128 * (j2 + 1)],
                                x_tiles[k][:, 512 * j:512 * (j + 1)],
                                start=(k == 0),
                                stop=(k == KT - 1),
                            )
                for j2 in range(2):
                    n_sub = 2 * pair + j2
                    for j in range(MCH):
                        ot = ev_pool.tile([P, 512], F32, tag="ot")
                        nc.scalar.activation(
                            ot[:],
                            psums[j2][j][:],
                            mybir.ActivationFunctionType.Identity,
                            bias=b_tiles[n_sub][:],
                        )
                        nc.sync.dma_start(
                            outT[n_sub * P:(n_sub + 1) * P, 512 * j:512 * (j + 1)],
                            ot[:],
                        )
    nc.compile()
    return nc


def make_in_maps(input, weight, bias):
    x = np.asarray(input, dtype=np.float32)
    w = np.asarray(weight, dtype=np.float32)
    b = np.asarray(bias, dtype=np.float32)
    wT = np.ascontiguousarray(w.T)
    bcol = np.ascontiguousarray(b.reshape(N, 1))
    in_maps = []
    for c in range(NCORES):
        xTc = np.ascontiguousarray(x[c * BC:(c + 1) * BC, :].T)
        in_maps.append({"xT": xTc, "wT": wT, "bias": bcol})
    return in_maps


def gather(results):
    out = np.empty((B, N), dtype=np.float32)
    for c in range(NCORES):
        out[c * BC:(c + 1) * BC, :] = results[c]["outT"].T
    return out


def kernel(input, weight, bias):
    if "nc" not in _cached:
        _cached["nc"] = build()
    nc = _cached["nc"]
    in_maps = make_in_maps(input, weight, bias)
    res = run_bass_kernel_spmd(nc, in_maps, core_ids=list(range(NCORES)))
    return gather(res.results)


# revision 9
# speedup vs baseline: 1.0892x; 1.0892x over previous
"""TRN2 Bass kernel for nn_CudaSafeLinear: out = input @ weight.T + bias.

Shapes: input [8192, 4096] f32, weight [4096, 4096] f32, bias [4096] f32.
Sharding: data-parallel over batch rows — core c computes rows [1024c, 1024(c+1)).

Per-core GEMM (out^T orientation):
  outT[n, m] = sum_k wT[k, n] * xT[k, m] + bias[n]
with wT = weight.T ([K, N] in DRAM), xT = input_shard.T ([K, 1024]).
Stationary operand = wT k-tiles [128, 128]; moving operand = resident xT
chunks [128, 512]. Matmuls run in float32r (TF32-class precision, full PE
rate at moving dim >= 256). Accumulation is fp32 in PSUM; bias is added on
the Scalar engine during PSUM->SBUF eviction (psum partitions = out
features, so bias is a per-partition scalar).
"""

import numpy as np

import concourse.mybir as mybir
import concourse.tile as tile
from concourse import bacc
from concourse.bass_utils import run_bass_kernel_spmd

B, K, N = 8192, 4096, 4096
NCORES = 8
BC = B // NCORES          # 1024 batch rows per core
P = 128
KT = K // P               # 32 contraction tiles
MCH = BC // 512           # 2 moving chunks of 512
NSUB = N // P             # 32 stationary (out-feature) tiles
F32R = mybir.dt.float32r
F32 = mybir.dt.float32

_cached = {}


def build():
    nc = bacc.Bacc("TRN2", target_bir_lowering=False, debug=False, num_devices=NCORES)
    xT = nc.dram_tensor("xT", [K, BC], F32R, kind="ExternalInput").ap()
    wT = nc.dram_tensor("wT", [K, N], F32R, kind="ExternalInput").ap()
    bias = nc.dram_tensor("bias", [N, 1], F32, kind="ExternalInput").ap()
    outT = nc.dram_tensor("outT", [N, BC], F32, kind="ExternalOutput").ap()

    with tile.TileContext(nc) as tc:
        with (
            tc.tile_pool(name="xres", bufs=1) as x_pool,
            tc.tile_pool(name="bres", bufs=1) as b_pool,
            tc.tile_pool(name="w", bufs=12) as w_pool,
            tc.tile_pool(name="ps", bufs=8, space="PSUM") as ps_pool,
            tc.tile_pool(name="ev", bufs=4) as ev_pool,
        ):
            # Resident input shard: 32 k-tiles of [128, 1024] f32r (16.8 MB).
            # Split across two engines' HW-DGE queues so the load runs at
            # ~2x single-queue bandwidth and doesn't gate compute start.
            x_tiles = []
            for k in range(KT):
                xt = x_pool.tile([P, BC], F32R, tag=f"x{k}")
                eng = nc.gpsimd if k % 2 == 0 else nc.scalar
                eng.dma_start(xt[:], xT[k * P:(k + 1) * P, :])
                x_tiles.append(xt)
            # Resident bias: [128, 1] per out-feature tile.
            b_tiles = []
            for i in range(NSUB):
                bt = b_pool.tile([P, 1], F32, tag=f"b{i}")
                nc.gpsimd.dma_start(bt[:], bias[i * P:(i + 1) * P, :])
                b_tiles.append(bt)

            for pair in range(NSUB // 2):  # two n_subs share each weight DMA
                psums = [
                    [ps_pool.tile([P, 512], F32, tag="ps", name="ps") for _ in range(MCH)]
                    for _ in range(2)
                ]
                for k in range(KT):
                    wt = w_pool.tile([P, 256], F32R, tag="w")
                    nc.sync.dma_start(
                        wt[:], wT[k * P:(k + 1) * P, 256 * pair:256 * (pair + 1)]
                    )
                    for j2 in range(2):
                        for j in range(MCH):
                            nc.tensor.matmul(
                                psums[j2][j][:],
                                wt[:, 128 * j2:128 * (j2 + 1)],
                                x_tiles[k][:, 512 * j:512 * (j + 1)],
                                start=(k == 0),
                                stop=(k == KT - 1),
                            )
                for j2 in range(2):
                    n_sub = 2 * pair + j2
                    for j in range(MCH):
                        ot = ev_pool.tile([P, 512], F32, tag="ot")
                        nc.scalar.activation(
                            ot[:],
                            psums[j2][j][:],
                            mybir.ActivationFunctionType.Identity,
                            bias=b_tiles[n_sub][:],
                        )
                        nc.scalar.dma_start(
                            outT[n_sub * P:(n_sub + 1) * P, 512 * j:512 * (j + 1)],
                            ot[:],
                        )
    nc.compile()
    return nc


def make_in_maps(input, weight, bias):
    x = np.asarray(input, dtype=np.float32)
    w = np.asarray(weight, dtype=np.float32)
    b = np.asarray(bias, dtype=np.float32)
    wT = np.ascontiguousarray(w.T)
    bcol = np.ascontiguousarray(b.reshape(N, 1))
    in_maps = []
    for c in range(NCORES):
        xTc = np.ascontiguousarray(x[c * BC:(c + 1) * BC, :].T)
        in_maps.append({"xT": xTc, "wT": wT, "bias": bcol})
    return in_maps


def gather(results):
    out = np.empty((B, N), dtype=np.float32)
    for c in range(NCORES):
        out[c * BC:(c + 1) * BC, :] = results[c]["outT"].T
    return out


def kernel(input, weight, bias):
    if "nc" not in _cached:
        _cached["nc"] = build()
    nc = _cached["nc"]
    in_maps = make_in_maps(input, weight, bias)
    res = run_bass_kernel_spmd(nc, in_maps, core_ids=list(range(NCORES)))
    return gather(res.results)
